# revision 24
# baseline (speedup 1.0000x reference)
"""Trainium2 Bass kernel for the bipartite GNN message-passing encoder.

Math:
  A_r = (adj == r), r = 1..5
  An_r = diag(a) A_r diag(b),  a = 1/sqrt(Nu), b = 1/sqrt(Nv)
  Hu = relu(sum_r An_r @ W_items_r^T)   [NU, M]
  Hv = relu(sum_r An_r^T @ W_users_r^T) [NI, M]
  U  = relu(Hu @ dense_W^T + relu(u_sf @ u_W1^T + u_b1) @ u_W2^T)
  V  = relu(Hv @ dense_W^T + relu(v_sf @ v_W1^T + v_b1) @ v_W2^T)

Sharding: fully collective-free 1D row split per bipartite side. Core c
owns users [500c, 500c+500) and items [500c, 500c+500) and contracts
over the FULL opposite side locally, so no partial-sum AllReduce is
ever needed. Degree normalizations are folded on the host (inner-side
factor into the message weights, outer-side factor as a per-partition
scale in pass 2).

Speed comes from fp8 DoubleRow matmuls (2 k-tiles contracted per PE
instruction at ~259ns for 512 moving columns => ~2x bf16 MACs). Both
operands must be fp8e4, so the rating one-hot expansion is
restructured around cheap fp8 streams:
  sum_r M_r W_r = A (W_4/4) + sum_{r in 1,2,3,5} M_r (W_r - r/4 W_4)
using M_4 = (A - 1M_1 - 2M_2 - 3M_3 - 5M_5)/4. The A stream is the raw
adjacency (ints 0..5, exact in fp8) needing NO mask op; M_5 comes
precomputed from the host (DMA is cheap, DVE is not); only M_1..M_3
are built on-device (DVE is_equal, fp8-out runs at 2 elem/cycle/lane
so 3 masks/side ~30us fits under the ~41us/side PE stream). Weights
are scaled by S=2048 on host so the b-folded values clear fp8e4m3's
subnormal floor; 1/S is folded into dense_W.

Layouts: contraction k-tiles are paired for DoubleRow ([128, 2, .]
APs, pair stride %16==0 => moving blocks padded 500->512, PSUM tile
[128,512] = exactly one bank). W per k-pair: [128, 2, 5*256], column
order [T_A, T_1, T_2, T_3, T_5]. adj/M5 arrive in 2-k-pair granules so
one DVE op builds 2 k-pairs of one rating's mask.

Schedule: granule resources (DMAs then masks) are emitted PD=4
granules AHEAD of their matmuls, so the queues and the DVE run well in
front of the PE and ride out HBM-contention bursts (8 cores share the
stacks; transient squeezes of one queue are real). Streams are placed
the whole PE-gating chain (adj then W, ~200 GB/s) rides the ONE
measured-strong queue (scalar, 250+ GB/s) in consumption order -- a
stream on a second queue gets squeezed when the strong queue bursts
into the shared HBM ceiling. M5 rides gpsimd/SWDGE; sync stays light. Side B's granule 0
(adj/M5/W k-pairs 0-1 + masks) and the side weights are prefetched
while side A's last granules compute, which collapses the A->B
boundary to the eviction+projection latency.
"""

import sys

import numpy as np

if "/opt/trn_rl_repo" not in sys.path:
    sys.path.insert(0, "/opt/trn_rl_repo")

import concourse.bacc as bacc  # noqa: E402
import concourse.mybir as mybir  # noqa: E402
import concourse.tile as tile  # noqa: E402

FP = mybir.dt.float32
BF = mybir.dt.bfloat16
F8 = mybir.dt.float8e4

NU = NI = 4000
R = 5
M = 256
OUT = 75
SIDE = 64
FDIM = 128

NCORES = 8
B = NU // NCORES  # 500 output rows per side per core
BP = 512  # moving block padded (DoubleRow pair stride must be %16==0)
NP = 4096  # contraction dim padded to a multiple of 256
KT = NP // 128  # 32 k-copies
KP = KT // 2  # 16 DoubleRow k-pairs
NG = KP // 2  # 8 two-k-pair granules (one DVE mask op per rating each)
NS = 5  # fp8 streams per k-pair: A, M1, M2, M3, M5
WREC = NS * M  # 1280 W columns per k-copy
CH = B // 4  # 125-row output chunks in pass 2
SSCALE = 2048.0  # fp8 weight scale (power of 2; 1/S folded into dense_W)
DVE_R = (1, 2, 3)  # ratings whose masks are built on-device
PD = 4  # granule prefetch depth: queues run PD granules ahead of the PE

# byte layout of the batched bf16 side-weight tensor [128, SBF_F]
SBF_UFT = 0
SBF_VFT = SBF_UFT + B
SBF_UW1 = SBF_VFT + B
SBF_VW1 = SBF_UW1 + SIDE
SBF_DW0 = SBF_VW1 + SIDE
SBF_DW1 = SBF_DW0 + OUT
SBF_UW2 = SBF_DW1 + OUT
SBF_VW2 = SBF_UW2 + OUT
SBF_F = SBF_VW2 + OUT
# and the f32 one [128, SFP_F]: ub1, vb1, afac(4), bfac(4)
SFP_F = 10

AF = mybir.ActivationFunctionType
ALU = mybir.AluOpType
PM = mybir.MatmulPerfMode


def build_program():
    from contextlib import ExitStack

    nc = bacc.Bacc("TRN2", target_bir_lowering=False, debug=False, num_devices=NCORES)

    adjtu = nc.dram_tensor("adjtu", [128, KT, BP], F8, kind="ExternalInput")
    adjv = nc.dram_tensor("adjv", [128, KT, BP], F8, kind="ExternalInput")
    m5tu = nc.dram_tensor("m5tu", [128, KT, BP], F8, kind="ExternalInput")
    m5v = nc.dram_tensor("m5v", [128, KT, BP], F8, kind="ExternalInput")
    wi = nc.dram_tensor("wi", [128, KT, WREC], F8, kind="ExternalInput")
    wu = nc.dram_tensor("wu", [128, KT, WREC], F8, kind="ExternalInput")
    sbf_d = nc.dram_tensor("sbf", [128, SBF_F], BF, kind="ExternalInput")
    sfp_d = nc.dram_tensor("sfp", [128, SFP_F], FP, kind="ExternalInput")
    u_out = nc.dram_tensor("u_out", [CH, 4, OUT], FP, kind="ExternalOutput")
    v_out = nc.dram_tensor("v_out", [CH, 4, OUT], FP, kind="ExternalOutput")

    with tile.TileContext(nc) as tc, ExitStack() as ctx:
        res = ctx.enter_context(tc.tile_pool(name="res", bufs=1))
        wpool = ctx.enter_context(tc.tile_pool(name="wpool", bufs=7))
        spool = ctx.enter_context(tc.tile_pool(name="spool", bufs=7))
        apool = m5pool = mpool = spool
        scr = ctx.enter_context(tc.tile_pool(name="scr", bufs=2))
        # PSUM budget (8 banks): psA 2 + psB 2 + psf 2 (released before
        # pass 2 opens ps2's 4)
        psA = ctx.enter_context(tc.tile_pool(name="psA", bufs=1, space="PSUM"))
        psB = ctx.enter_context(tc.tile_pool(name="psB", bufs=1, space="PSUM"))
        psf = tc.alloc_tile_pool(name="psf", bufs=1, space="PSUM")

        # ---- one granule's worth of pass-1 inputs: DMAs first (adj on
        # scalar, M5 on gpsimd, W even k-pair on sync / odd on vector so two
        # HW queues split the heavy stream), then the DVE mask builds ----
        def emit_granule(adj_dram, m5_dram, w_dram, g, first_masks=False):
            # k-pair-granular DMAs (slice-level dependency tracking lets the
            # first matmuls gate on 455KB, not 910KB -- early-phase DMA runs
            # ~2x slower); during side A's ramp the second k-pair's W rides
            # the otherwise-idle sync queue for extra early bandwidth
            at = apool.tile([128, 4, BP], F8, tag="adjg", name="at")
            wt = wpool.tile([128, 4, WREC], F8, tag="wt", name="wt")
            if first_masks:
                for j in range(2):
                    nc.scalar.dma_start(
                        out=at[:, 2 * j : 2 * j + 2, :],
                        in_=adj_dram[:, 4 * g + 2 * j : 4 * g + 2 * j + 2, :],
                    )
                    nc.scalar.dma_start(
                        out=wt[:, 2 * j : 2 * j + 2, :],
                        in_=w_dram[:, 4 * g + 2 * j : 4 * g + 2 * j + 2, :],
                    )
            else:
                nc.scalar.dma_start(
                    out=at[:, :, :], in_=adj_dram[:, 4 * g : 4 * g + 4, :]
                )
                nc.scalar.dma_start(
                    out=wt[:, :, :], in_=w_dram[:, 4 * g : 4 * g + 4, :]
                )
            m5t = m5pool.tile([128, 4, BP], F8, tag="m5g", name="m5t")
            nc.gpsimd.dma_start(
                out=m5t[:, :, :], in_=m5_dram[:, 4 * g : 4 * g + 4, :]
            )
            masks = {
                r: mpool.tile([128, 4, BP], F8, tag=f"msk{r}", name="msk")
                for r in DVE_R
            }
            if first_masks:  # per-k-pair halves: halves the first-mask latency
                for j in range(2):
                    for r in DVE_R:
                        nc.vector.tensor_scalar(
                            out=masks[r][:, 2 * j : 2 * j + 2, :],
                            in0=at[:, 2 * j : 2 * j + 2, :],
                            scalar1=float(r), scalar2=None, op0=ALU.is_equal,
                        )
            else:
                for r in DVE_R:
                    nc.vector.tensor_scalar(
                        out=masks[r][:, :, :], in0=at[:, :, :],
                        scalar1=float(r), scalar2=None, op0=ALU.is_equal,
                    )
            return {"at": at, "m5t": m5t, "wt": wt, "masks": masks}

        # ---- pass 1: fp8 DoubleRow stream, resources emitted 1 granule
        # ahead of the matmuls so queues+DVE lead the PE ----
        def pass1(adj_dram, m5_dram, w_dram, pspool, nm, pre=None,
                  prefetch_last=None):
            # moving dim is 500 of the 512-stride padded blocks: the
            # DoubleRow %16 constraint binds the PAIR STRIDE, not the width
            pst = [
                pspool.tile([128, B], FP, tag=f"ps{nm}{mt}", name=f"ps{nm}")
                for mt in range(2)
            ]
            tiles = {0: pre if pre is not None else
                     emit_granule(adj_dram, m5_dram, w_dram, 0, first_masks=True)}
            for gg in range(1, PD):
                tiles[gg] = emit_granule(adj_dram, m5_dram, w_dram, gg)
            for g in range(NG):
                if g + PD < NG:
                    tiles[g + PD] = emit_granule(adj_dram, m5_dram, w_dram, g + PD)
                elif g + PD == NG and prefetch_last is not None:
                    prefetch_last()
                t = tiles.pop(g)
                for j in range(2):
                    kp = 2 * g + j
                    wt = t["wt"]
                    rhss = [
                        t["at"][:, 2 * j : 2 * j + 2, :B],
                        t["masks"][1][:, 2 * j : 2 * j + 2, :B],
                        t["masks"][2][:, 2 * j : 2 * j + 2, :B],
                        t["masks"][3][:, 2 * j : 2 * j + 2, :B],
                        t["m5t"][:, 2 * j : 2 * j + 2, :B],
                    ]
                    for s, rhs in enumerate(rhss):
                        for mt in range(2):
                            wof = s * M + mt * 128
                            nc.tensor.matmul(
                                pst[mt][:, :],
                                lhsT=wt[:, 2 * j : 2 * j + 2, wof : wof + 128],
                                rhs=rhs,
                                start=(kp == 0 and s == 0),
                                stop=(kp == KP - 1 and s == NS - 1),
                                perf_mode=PM.DoubleRow,
                            )
            return pst

        def evict(pst, nm):
            hT = [
                res.tile([128, B], BF, tag=f"h{nm}T{mt}", name=f"h{nm}T{mt}")
                for mt in range(2)
            ]
            for half in range(2):
                lo, hi = half * (B // 2), (half + 1) * (B // 2)
                for mt in range(2):
                    nc.scalar.activation(
                        out=hT[mt][:, lo:hi], in_=pst[mt][:, lo:hi], func=AF.Relu
                    )
            return hT

        # ---- batched side weights: two DMAs instead of thirteen ----
        def load_smalls():
            sbf = res.tile([128, SBF_F], BF, tag="sbf")
            nc.scalar.dma_start(out=sbf[:, :], in_=sbf_d[:, :])
            sfp = res.tile([128, SFP_F], FP, tag="sfp")
            nc.scalar.dma_start(out=sfp[:, :], in_=sfp_d[:, :])
            return {
                "uft": sbf[:, SBF_UFT : SBF_UFT + B],
                "vft": sbf[:, SBF_VFT : SBF_VFT + B],
                "uw1t": sbf[:, SBF_UW1 : SBF_UW1 + SIDE],
                "vw1t": sbf[:, SBF_VW1 : SBF_VW1 + SIDE],
                "dwt": [sbf[:, SBF_DW0 : SBF_DW0 + OUT],
                        sbf[:, SBF_DW1 : SBF_DW1 + OUT]],
                "uw2t": sbf[0:SIDE, SBF_UW2 : SBF_UW2 + OUT],
                "vw2t": sbf[0:SIDE, SBF_VW2 : SBF_VW2 + OUT],
                "ub1": sfp[0:SIDE, 0:1],
                "vb1": sfp[0:SIDE, 1:2],
                "afac": sfp[0:CH, 2:6],
                "bfac": sfp[0:CH, 6:10],
            }

        # ---- side-feature projections (PE-tiny; emitted at the boundary) ---
        def side_proj(sf_t, w1_t, b1_t, tag):
            p = psf.tile([SIDE, B], FP, tag=f"psf{tag}", name="psf")
            nc.tensor.matmul(
                p[:, :], lhsT=w1_t[:FDIM, :SIDE], rhs=sf_t[:FDIM, :B],
                start=True, stop=True,
            )
            fT = res.tile([SIDE, B], BF, tag=f"fT{tag}")
            nc.scalar.activation(
                out=fT[:, :], in_=p[:, :], func=AF.Relu, bias=b1_t[:, :]
            )
            return fT

        # ---- pass 2: dense head + side head per 125-row chunk; relu and
        # the store batched over all 4 chunks ----
        def pass2(hT, fT, w2t_t, fac_t, dwt_t, o_dram, nm, out_eng):
            soT = scr.tile([CH, 4, OUT], FP, tag="soT", name="soT")
            ro = scr.tile([CH, 4, OUT], FP, tag="ro", name="ro")
            for c in range(4):
                pd = ps2.tile([CH, OUT], FP, tag="pd", name="pd")
                for mt in range(2):
                    nc.tensor.matmul(
                        pd[:, :], lhsT=hT[mt][:, c * CH : (c + 1) * CH],
                        rhs=dwt_t[mt][:, :OUT],
                        start=(mt == 0), stop=(mt == 1),
                    )
                ps_ = ps2.tile([CH, OUT], FP, tag="pss", name="ps_")
                nc.tensor.matmul(
                    ps_[:, :], lhsT=fT[:SIDE, c * CH : (c + 1) * CH],
                    rhs=w2t_t[:SIDE, :OUT], start=True, stop=True,
                )
                sa = scr.tile([CH, OUT], FP, tag="sa", name="sa")
                nc.vector.tensor_scalar(
                    out=sa[:, :], in0=pd[:, :], scalar1=fac_t[:, c : c + 1],
                    scalar2=None, op0=ALU.mult,
                )
                nc.vector.tensor_tensor(
                    out=soT[:, c, :], in0=ps_[:, :], in1=sa[:, :], op=ALU.add
                )
                if c % 2 == 1:  # relu+store per half: first store starts early
                    h0 = c - 1
                    nc.scalar.activation(
                        out=ro[:, h0 : c + 1, :], in_=soT[:, h0 : c + 1, :],
                        func=AF.Relu,
                    )
                    out_eng.dma_start(
                        out=o_dram[:, h0 : c + 1, :], in_=ro[:, h0 : c + 1, :]
                    )

        # PE keep-alive: dummy matmuls that hold the HAM clock at full speed
        # while short ACT/DVE chains drain (results never read)
        # The dummy target tag aliases a pass-1 accumulator buffer that is
        # dead (already evicted, or not yet started: start=True resets it).
        def dummy_mms(n, src, pool, tag):
            w = min(B, src.shape[-1])
            for _ in range(n):
                t = pool.tile([128, B], FP, tag=tag, name="dmy")
                nc.tensor.matmul(
                    t[:, :w], lhsT=src[:, :128], rhs=src[:, :w],
                    start=True, stop=True, skip_group_check=True,
                )

        # PE warm-up: dummy matmuls on a memset tile pull the HAM clock to
        # 2.4 GHz while the first granule's DMAs land
        warm = res.tile([128, BP], BF, tag="warm")
        nc.gpsimd.memset(warm[:, :], 0.0)
        dummy_mms(15, warm, psA, "psA0")

        # Emission order: side A stream (side B granule 0 + side weights
        # prefetched during A's last granule); boundary = evictions + side
        # projections + side A pass 2 (overlaps side B's ramp); side B
        # stream; tiny tail.
        preB = {}
        sm = {}

        def prefetch_boundary():
            preB.update(emit_granule(adjv, m5v, wu, 0))
            sm.update(load_smalls())

        pstA = pass1(adjtu, m5tu, wi, psA, "A", prefetch_last=prefetch_boundary)
        dummy_mms(5, warm, psB, "psB0")  # cover the eviction latency
        huT = evict(pstA, "u")
        fuT = side_proj(sm["uft"], sm["uw1t"], sm["ub1"], "u")
        fvT = side_proj(sm["vft"], sm["vw1t"], sm["vb1"], "v")
        psf.release()
        ps2 = tc.alloc_tile_pool(name="ps2", bufs=2, space="PSUM")
        pass2(huT, fuT, sm["uw2t"], sm["afac"], sm["dwt"], u_out, "u", nc.scalar)
        pstB = pass1(adjv, m5v, wu, psB, "B", pre=preB)
        hvT = evict(pstB, "v")
        dummy_mms(6, huT[0], psA, "psA0")
        pass2(hvT, fvT, sm["vw2t"], sm["bfac"], sm["dwt"], v_out, "v", nc.scalar)
        dummy_mms(4, huT[0], psA, "psA0")
        ps2.release()

    nc.compile()
    return nc


_CACHE = {}


def _get_program():
    if "nc" not in _CACHE:
        _CACHE["nc"] = build_program()
    return _CACHE["nc"]


def make_in_maps(inputs):
    import ml_dtypes

    bf16 = ml_dtypes.bfloat16
    f8 = ml_dtypes.float8_e4m3

    adj = np.asarray(inputs["adj_matrix"], dtype=np.int32)
    u_sf = np.asarray(inputs["u_sideFeat"], dtype=np.float32)
    v_sf = np.asarray(inputs["v_sideFeat"], dtype=np.float32)
    msg_W = np.asarray(inputs["msg_W"], dtype=np.float64)
    dense_W = np.asarray(inputs["dense_W"], dtype=np.float32)
    u_W1 = np.asarray(inputs["u_W1"], dtype=np.float32)
    u_b1 = np.asarray(inputs["u_b1"], dtype=np.float32)
    u_W2 = np.asarray(inputs["u_W2"], dtype=np.float32)
    v_W1 = np.asarray(inputs["v_W1"], dtype=np.float32)
    v_b1 = np.asarray(inputs["v_b1"], dtype=np.float32)
    v_W2 = np.asarray(inputs["v_W2"], dtype=np.float32)

    # degree normalization (exact, f64); Csafe guard only matters off-support
    nz = adj != 0
    a = 1.0 / np.sqrt(np.maximum(nz.sum(axis=1), 1))  # [NU]
    b = 1.0 / np.sqrt(np.maximum(nz.sum(axis=0), 1))  # [NI]

    # fp8 stream weights. M_4 is eliminated via
    #   sum_r M_r W_r = A (W_4/4) + sum_{r!=4} M_r (W_r - r/4 W_4)
    # then the contraction-side degree factor and S are folded in, and the
    # result laid out k-copy-major for the DoubleRow lhsT APs:
    #   [p, k, s*256 + m] = T_s^T[128k+p, m],  s order [A, M1, M2, M3, M5]
    def w_stream(wT, fold):  # wT [R, M, 4000] -> [128, KT, WREC] fp8
        t = np.empty((NS, M, NU), np.float64)
        w4 = wT[3]
        t[0] = w4 / 4.0
        for si, r in enumerate((1, 2, 3, 5)):
            t[1 + si] = wT[r - 1] - (r / 4.0) * w4
        t *= fold[None, None, :] * SSCALE
        tp = np.zeros((NP, NS * M), np.float64)
        tp[:NU, :] = t.transpose(2, 0, 1).reshape(NU, NS * M)
        return np.ascontiguousarray(
            tp.reshape(KT, 128, WREC).transpose(1, 0, 2)
        ).astype(f8)

    wi_s = w_stream(msg_W[:, :, NU:], b)
    wu_s = w_stream(msg_W[:, :, :NU], a)

    # adjacency / M5 blocks, padded, k-copy-major: [128, KT, BP] per core
    def blocks(m):  # m [4000(contraction), 4000(moving)] -> [128, KT, 4000]
        mp = np.zeros((NP, NU), np.float32)
        mp[:NU, :] = m
        return mp.reshape(KT, 128, NU).transpose(1, 0, 2)

    adjv_b = blocks(adj)  # contraction over users (rows)
    adjtu_b = blocks(adj.T)  # contraction over items
    m5v_b = blocks((adj == 5).astype(np.float32))
    m5tu_b = blocks((adj.T == 5).astype(np.float32))

    def core_slice(bl, s):  # [128, KT, 4000] -> [128, KT, BP] fp8
        out = np.zeros((128, KT, BP), f8)
        out[:, :, :B] = bl[:, :, s : s + B].astype(f8)
        return out

    # batched side-weight tensors (bf16 at 128 partitions + f32 scalars)
    uftT = u_sf.T.astype(bf16)  # [FDIM, NU]
    vftT = v_sf.T.astype(bf16)
    sbf_shared = np.zeros((128, SBF_F), bf16)
    sbf_shared[:FDIM, SBF_UW1 : SBF_UW1 + SIDE] = u_W1.T.astype(bf16)
    sbf_shared[:FDIM, SBF_VW1 : SBF_VW1 + SIDE] = v_W1.T.astype(bf16)
    dwt = (dense_W.T / SSCALE).astype(bf16)  # [M, OUT]
    sbf_shared[:, SBF_DW0 : SBF_DW0 + OUT] = dwt[:128]
    sbf_shared[:, SBF_DW1 : SBF_DW1 + OUT] = dwt[128:]
    sbf_shared[:SIDE, SBF_UW2 : SBF_UW2 + OUT] = u_W2.T.astype(bf16)
    sbf_shared[:SIDE, SBF_VW2 : SBF_VW2 + OUT] = v_W2.T.astype(bf16)

    def chunked(v):  # [B] f64 -> [CH, 4] f32 column-per-chunk
        return np.ascontiguousarray(v.reshape(4, CH).T).astype(np.float32)

    in_maps = []
    for c in range(NCORES):
        s = c * B
        sbf = sbf_shared.copy()
        sbf[:FDIM, SBF_UFT : SBF_UFT + B] = uftT[:, s : s + B]
        sbf[:FDIM, SBF_VFT : SBF_VFT + B] = vftT[:, s : s + B]
        sfp = np.zeros((128, SFP_F), np.float32)
        sfp[:SIDE, 0] = u_b1
        sfp[:SIDE, 1] = v_b1
        sfp[:CH, 2:6] = chunked(a[s : s + B])
        sfp[:CH, 6:10] = chunked(b[s : s + B])
        in_maps.append(
            {
                "wi": wi_s,
                "wu": wu_s,
                "adjtu": core_slice(adjtu_b, s),
                "adjv": core_slice(adjv_b, s),
                "m5tu": core_slice(m5tu_b, s),
                "m5v": core_slice(m5v_b, s),
                "sbf": sbf,
                "sfp": sfp,
            }
        )
    return in_maps


def assemble(results):
    U = np.empty((NU, OUT), np.float32)
    V = np.empty((NI, OUT), np.float32)
    for c in range(NCORES):
        U[c * B : (c + 1) * B] = (
            results[c]["u_out"].transpose(1, 0, 2).reshape(B, OUT)
        )
        V[c * B : (c + 1) * B] = (
            results[c]["v_out"].transpose(1, 0, 2).reshape(B, OUT)
        )
    return (U, V)


def kernel(**inputs):
    from concourse.bass_utils import run_bass_kernel_spmd

    nc = _get_program()
    res = run_bass_kernel_spmd(nc, make_in_maps(inputs), core_ids=list(range(NCORES)))
    return assemble(res.results)


# revision 25
# speedup vs baseline: 1.1765x; 1.1765x over previous
"""Trainium2 Bass kernel for the bipartite GNN message-passing encoder.

Math:
  A_r = (adj == r), r = 1..5
  An_r = diag(a) A_r diag(b),  a = 1/sqrt(Nu), b = 1/sqrt(Nv)
  Hu = relu(sum_r An_r @ W_items_r^T)   [NU, M]
  Hv = relu(sum_r An_r^T @ W_users_r^T) [NI, M]
  U  = relu(Hu @ dense_W^T + relu(u_sf @ u_W1^T + u_b1) @ u_W2^T)
  V  = relu(Hv @ dense_W^T + relu(v_sf @ v_W1^T + v_b1) @ v_W2^T)

Sharding: fully collective-free 1D row split per bipartite side. Core c
owns users [500c, 500c+500) and items [500c, 500c+500) and contracts
over the FULL opposite side locally, so no partial-sum AllReduce is
ever needed. Degree normalizations are folded on the host (inner-side
factor into the message weights, outer-side factor as a per-partition
scale in pass 2).

Speed comes from fp8 DoubleRow matmuls (2 k-tiles contracted per PE
instruction at ~259ns for 512 moving columns => ~2x bf16 MACs). Both
operands must be fp8e4, so the rating one-hot expansion is
restructured around cheap fp8 streams:
  sum_r M_r W_r = A (W_4/4) + sum_{r in 1,2,3,5} M_r (W_r - r/4 W_4)
using M_4 = (A - 1M_1 - 2M_2 - 3M_3 - 5M_5)/4. The A stream is the raw
adjacency (ints 0..5, exact in fp8) needing NO mask op; M_5 comes
precomputed from the host (DMA is cheap, DVE is not); only M_1..M_3
are built on-device (DVE is_equal, fp8-out runs at 2 elem/cycle/lane
so 3 masks/side ~30us fits under the ~41us/side PE stream). Weights
are scaled by S=2048 on host so the b-folded values clear fp8e4m3's
subnormal floor; 1/S is folded into dense_W.

Layouts: contraction k-tiles are paired for DoubleRow ([128, 2, .]
APs, pair stride %16==0 => moving blocks padded 500->512, PSUM tile
[128,512] = exactly one bank). W per k-pair: [128, 2, 5*256], column
order [T_A, T_1, T_2, T_3, T_5]. adj/M5 arrive in 2-k-pair granules so
one DVE op builds 2 k-pairs of one rating's mask.

Schedule: granule resources (DMAs then masks) are emitted PD=4
granules AHEAD of their matmuls, so the queues and the DVE run well in
front of the PE and ride out HBM-contention bursts (8 cores share the
stacks; transient squeezes of one queue are real). Streams are placed
the whole PE-gating chain (adj then W, ~200 GB/s) rides the ONE
measured-strong queue (scalar, 250+ GB/s) in consumption order -- a
stream on a second queue gets squeezed when the strong queue bursts
into the shared HBM ceiling. M5 rides gpsimd/SWDGE; sync stays light. Side B's granule 0
(adj/M5/W k-pairs 0-1 + masks) and the side weights are prefetched
while side A's last granules compute, which collapses the A->B
boundary to the eviction+projection latency.
"""

import sys

import numpy as np

if "/opt/trn_rl_repo" not in sys.path:
    sys.path.insert(0, "/opt/trn_rl_repo")

import concourse.bacc as bacc  # noqa: E402
import concourse.mybir as mybir  # noqa: E402
import concourse.tile as tile  # noqa: E402

FP = mybir.dt.float32
BF = mybir.dt.bfloat16
F8 = mybir.dt.float8e4

NU = NI = 4000
R = 5
M = 256
OUT = 75
SIDE = 64
FDIM = 128

NCORES = 8
B = NU // NCORES  # 500 output rows per side per core
BP = 512  # moving block padded (DoubleRow pair stride must be %16==0)
NP = 4096  # contraction dim padded to a multiple of 256
KT = NP // 128  # 32 k-copies
KP = KT // 2  # 16 DoubleRow k-pairs
NG = KP // 2  # 8 two-k-pair granules (one DVE mask op per rating each)
NS = 5  # fp8 streams per k-pair: A, M1, M2, M3, M5
WREC = NS * M  # 1280 W columns per k-copy
CH = B // 4  # 125-row output chunks in pass 2
SSCALE = 2048.0  # fp8 weight scale (power of 2; 1/S folded into dense_W)
DVE_R = (1, 2, 3)  # ratings whose masks are built on-device
PD = 4  # granule prefetch depth: queues run PD granules ahead of the PE

# byte layout of the batched bf16 side-weight tensor [128, SBF_F]
SBF_UFT = 0
SBF_VFT = SBF_UFT + B
SBF_UW1 = SBF_VFT + B
SBF_VW1 = SBF_UW1 + SIDE
SBF_DW0 = SBF_VW1 + SIDE
SBF_DW1 = SBF_DW0 + OUT
SBF_UW2 = SBF_DW1 + OUT
SBF_VW2 = SBF_UW2 + OUT
SBF_F = SBF_VW2 + OUT
# and the f32 one [128, SFP_F]: ub1, vb1, afac(4), bfac(4)
SFP_F = 10

AF = mybir.ActivationFunctionType
ALU = mybir.AluOpType
PM = mybir.MatmulPerfMode


def build_program():
    from contextlib import ExitStack

    nc = bacc.Bacc("TRN2", target_bir_lowering=False, debug=False, num_devices=NCORES)

    adjtu = nc.dram_tensor("adjtu", [128, KT, BP], F8, kind="ExternalInput")
    adjv = nc.dram_tensor("adjv", [128, KT, BP], F8, kind="ExternalInput")
    m5tu = nc.dram_tensor("m5tu", [128, KT, BP], F8, kind="ExternalInput")
    m5v = nc.dram_tensor("m5v", [128, KT, BP], F8, kind="ExternalInput")
    wi = nc.dram_tensor("wi", [128, KT, WREC], F8, kind="ExternalInput")
    wu = nc.dram_tensor("wu", [128, KT, WREC], F8, kind="ExternalInput")
    sbf_d = nc.dram_tensor("sbf", [128, SBF_F], BF, kind="ExternalInput")
    sfp_d = nc.dram_tensor("sfp", [128, SFP_F], FP, kind="ExternalInput")
    u_out = nc.dram_tensor("u_out", [CH, 4, OUT], FP, kind="ExternalOutput")
    v_out = nc.dram_tensor("v_out", [CH, 4, OUT], FP, kind="ExternalOutput")

    with tile.TileContext(nc) as tc, ExitStack() as ctx:
        res = ctx.enter_context(tc.tile_pool(name="res", bufs=1))
        wpool = ctx.enter_context(tc.tile_pool(name="wpool", bufs=7))
        apool = ctx.enter_context(tc.tile_pool(name="apool", bufs=7))
        m5pool = ctx.enter_context(tc.tile_pool(name="m5pool", bufs=7))
        mpool = ctx.enter_context(tc.tile_pool(name="mpool", bufs=7))
        scr = ctx.enter_context(tc.tile_pool(name="scr", bufs=2))
        # PSUM budget (8 banks): psA 2 + psB 2 + psf 2 (released before
        # pass 2 opens ps2's 4)
        psA = ctx.enter_context(tc.tile_pool(name="psA", bufs=1, space="PSUM"))
        psB = ctx.enter_context(tc.tile_pool(name="psB", bufs=1, space="PSUM"))
        psf = tc.alloc_tile_pool(name="psf", bufs=1, space="PSUM")

        # ---- one granule's worth of pass-1 inputs: DMAs first (adj on
        # scalar, M5 on gpsimd, W even k-pair on sync / odd on vector so two
        # HW queues split the heavy stream), then the DVE mask builds ----
        def emit_granule(adj_dram, m5_dram, w_dram, g, first_masks=False):
            # k-pair-granular DMAs (slice-level dependency tracking lets the
            # first matmuls gate on 455KB, not 910KB -- early-phase DMA runs
            # ~2x slower); during side A's ramp the second k-pair's W rides
            # the otherwise-idle sync queue for extra early bandwidth
            at = apool.tile([128, 4, BP], F8, tag="adjg", name="at")
            wt = wpool.tile([128, 4, WREC], F8, tag="wt", name="wt")
            if first_masks:
                for j in range(2):
                    nc.scalar.dma_start(
                        out=at[:, 2 * j : 2 * j + 2, :],
                        in_=adj_dram[:, 4 * g + 2 * j : 4 * g + 2 * j + 2, :],
                    )
                    nc.scalar.dma_start(
                        out=wt[:, 2 * j : 2 * j + 2, :],
                        in_=w_dram[:, 4 * g + 2 * j : 4 * g + 2 * j + 2, :],
                    )
            else:
                nc.scalar.dma_start(
                    out=at[:, :, :], in_=adj_dram[:, 4 * g : 4 * g + 4, :]
                )
                nc.scalar.dma_start(
                    out=wt[:, :, :], in_=w_dram[:, 4 * g : 4 * g + 4, :]
                )
            m5t = m5pool.tile([128, 4, BP], F8, tag="m5g", name="m5t")
            nc.gpsimd.dma_start(
                out=m5t[:, :, :], in_=m5_dram[:, 4 * g : 4 * g + 4, :]
            )
            masks = {
                r: mpool.tile([128, 4, BP], F8, tag=f"msk{r}", name="msk")
                for r in DVE_R
            }
            if first_masks:  # per-k-pair halves: halves the first-mask latency
                for j in range(2):
                    for r in DVE_R:
                        nc.vector.tensor_scalar(
                            out=masks[r][:, 2 * j : 2 * j + 2, :],
                            in0=at[:, 2 * j : 2 * j + 2, :],
                            scalar1=float(r), scalar2=None, op0=ALU.is_equal,
                        )
            else:
                for r in DVE_R:
                    nc.vector.tensor_scalar(
                        out=masks[r][:, :, :], in0=at[:, :, :],
                        scalar1=float(r), scalar2=None, op0=ALU.is_equal,
                    )
            return {"at": at, "m5t": m5t, "wt": wt, "masks": masks}

        # ---- pass 1: fp8 DoubleRow stream, resources emitted 1 granule
        # ahead of the matmuls so queues+DVE lead the PE ----
        def pass1(adj_dram, m5_dram, w_dram, pspool, nm, pre=None,
                  prefetch_last=None):
            # moving dim is 500 of the 512-stride padded blocks: the
            # DoubleRow %16 constraint binds the PAIR STRIDE, not the width
            pst = [
                pspool.tile([128, B], FP, tag=f"ps{nm}{mt}", name=f"ps{nm}")
                for mt in range(2)
            ]
            tiles = {0: pre if pre is not None else
                     emit_granule(adj_dram, m5_dram, w_dram, 0, first_masks=True)}
            for gg in range(1, PD):
                tiles[gg] = emit_granule(adj_dram, m5_dram, w_dram, gg)
            for g in range(NG):
                if g + PD < NG:
                    tiles[g + PD] = emit_granule(adj_dram, m5_dram, w_dram, g + PD)
                elif g + PD == NG and prefetch_last is not None:
                    prefetch_last()
                t = tiles.pop(g)
                for j in range(2):
                    kp = 2 * g + j
                    wt = t["wt"]
                    rhss = [
                        t["at"][:, 2 * j : 2 * j + 2, :B],
                        t["masks"][1][:, 2 * j : 2 * j + 2, :B],
                        t["masks"][2][:, 2 * j : 2 * j + 2, :B],
                        t["masks"][3][:, 2 * j : 2 * j + 2, :B],
                        t["m5t"][:, 2 * j : 2 * j + 2, :B],
                    ]
                    for s, rhs in enumerate(rhss):
                        for mt in range(2):
                            wof = s * M + mt * 128
                            nc.tensor.matmul(
                                pst[mt][:, :],
                                lhsT=wt[:, 2 * j : 2 * j + 2, wof : wof + 128],
                                rhs=rhs,
                                start=(kp == 0 and s == 0),
                                stop=(kp == KP - 1 and s == NS - 1),
                                perf_mode=PM.DoubleRow,
                            )
            return pst

        def evict(pst, nm):
            hT = [
                res.tile([128, B], BF, tag=f"h{nm}T{mt}", name=f"h{nm}T{mt}")
                for mt in range(2)
            ]
            for half in range(2):
                lo, hi = half * (B // 2), (half + 1) * (B // 2)
                for mt in range(2):
                    nc.scalar.activation(
                        out=hT[mt][:, lo:hi], in_=pst[mt][:, lo:hi], func=AF.Relu
                    )
            return hT

        # ---- batched side weights: two DMAs instead of thirteen ----
        def load_smalls():
            sbf = res.tile([128, SBF_F], BF, tag="sbf")
            nc.scalar.dma_start(out=sbf[:, :], in_=sbf_d[:, :])
            sfp = res.tile([128, SFP_F], FP, tag="sfp")
            nc.scalar.dma_start(out=sfp[:, :], in_=sfp_d[:, :])
            return {
                "uft": sbf[:, SBF_UFT : SBF_UFT + B],
                "vft": sbf[:, SBF_VFT : SBF_VFT + B],
                "uw1t": sbf[:, SBF_UW1 : SBF_UW1 + SIDE],
                "vw1t": sbf[:, SBF_VW1 : SBF_VW1 + SIDE],
                "dwt": [sbf[:, SBF_DW0 : SBF_DW0 + OUT],
                        sbf[:, SBF_DW1 : SBF_DW1 + OUT]],
                "uw2t": sbf[0:SIDE, SBF_UW2 : SBF_UW2 + OUT],
                "vw2t": sbf[0:SIDE, SBF_VW2 : SBF_VW2 + OUT],
                "ub1": sfp[0:SIDE, 0:1],
                "vb1": sfp[0:SIDE, 1:2],
                "afac": sfp[0:CH, 2:6],
                "bfac": sfp[0:CH, 6:10],
            }

        # ---- side-feature projections (PE-tiny; emitted at the boundary) ---
        def side_proj(sf_t, w1_t, b1_t, tag):
            p = psf.tile([SIDE, B], FP, tag=f"psf{tag}", name="psf")
            nc.tensor.matmul(
                p[:, :], lhsT=w1_t[:FDIM, :SIDE], rhs=sf_t[:FDIM, :B],
                start=True, stop=True,
            )
            fT = res.tile([SIDE, B], BF, tag=f"fT{tag}")
            nc.scalar.activation(
                out=fT[:, :], in_=p[:, :], func=AF.Relu, bias=b1_t[:, :]
            )
            return fT

        # ---- pass 2: dense head + side head per 125-row chunk; relu and
        # the store batched over all 4 chunks ----
        def pass2(hT, fT, w2t_t, fac_t, dwt_t, o_dram, nm, out_eng):
            soT = scr.tile([CH, 4, OUT], FP, tag="soT", name="soT")
            ro = scr.tile([CH, 4, OUT], FP, tag="ro", name="ro")
            for c in range(4):
                pd = ps2.tile([CH, OUT], FP, tag="pd", name="pd")
                for mt in range(2):
                    nc.tensor.matmul(
                        pd[:, :], lhsT=hT[mt][:, c * CH : (c + 1) * CH],
                        rhs=dwt_t[mt][:, :OUT],
                        start=(mt == 0), stop=(mt == 1),
                    )
                ps_ = ps2.tile([CH, OUT], FP, tag="pss", name="ps_")
                nc.tensor.matmul(
                    ps_[:, :], lhsT=fT[:SIDE, c * CH : (c + 1) * CH],
                    rhs=w2t_t[:SIDE, :OUT], start=True, stop=True,
                )
                sa = scr.tile([CH, OUT], FP, tag="sa", name="sa")
                nc.vector.tensor_scalar(
                    out=sa[:, :], in0=pd[:, :], scalar1=fac_t[:, c : c + 1],
                    scalar2=None, op0=ALU.mult,
                )
                nc.vector.tensor_tensor(
                    out=soT[:, c, :], in0=ps_[:, :], in1=sa[:, :], op=ALU.add
                )
                if c % 2 == 1:  # relu+store per half: first store starts early
                    h0 = c - 1
                    nc.scalar.activation(
                        out=ro[:, h0 : c + 1, :], in_=soT[:, h0 : c + 1, :],
                        func=AF.Relu,
                    )
                    out_eng.dma_start(
                        out=o_dram[:, h0 : c + 1, :], in_=ro[:, h0 : c + 1, :]
                    )

        # PE keep-alive: dummy matmuls that hold the HAM clock at full speed
        # while short ACT/DVE chains drain (results never read)
        # The dummy target tag aliases a pass-1 accumulator buffer that is
        # dead (already evicted, or not yet started: start=True resets it).
        def dummy_mms(n, src, pool, tag):
            w = min(B, src.shape[-1])
            for _ in range(n):
                t = pool.tile([128, B], FP, tag=tag, name="dmy")
                nc.tensor.matmul(
                    t[:, :w], lhsT=src[:, :128], rhs=src[:, :w],
                    start=True, stop=True, skip_group_check=True,
                )

        # PE warm-up: dummy matmuls on a memset tile pull the HAM clock to
        # 2.4 GHz while the first granule's DMAs land
        warm = res.tile([128, BP], BF, tag="warm")
        nc.gpsimd.memset(warm[:, :], 0.0)
        dummy_mms(15, warm, psA, "psA0")

        # Emission order: side A stream (side B granule 0 + side weights
        # prefetched during A's last granule); boundary = evictions + side
        # projections + side A pass 2 (overlaps side B's ramp); side B
        # stream; tiny tail.
        preB = {}
        sm = {}

        def prefetch_boundary():
            preB.update(emit_granule(adjv, m5v, wu, 0))
            sm.update(load_smalls())

        pstA = pass1(adjtu, m5tu, wi, psA, "A", prefetch_last=prefetch_boundary)
        dummy_mms(5, warm, psB, "psB0")  # cover the eviction latency
        huT = evict(pstA, "u")
        fuT = side_proj(sm["uft"], sm["uw1t"], sm["ub1"], "u")
        fvT = side_proj(sm["vft"], sm["vw1t"], sm["vb1"], "v")
        psf.release()
        ps2 = tc.alloc_tile_pool(name="ps2", bufs=2, space="PSUM")
        pass2(huT, fuT, sm["uw2t"], sm["afac"], sm["dwt"], u_out, "u", nc.scalar)
        pstB = pass1(adjv, m5v, wu, psB, "B", pre=preB)
        hvT = evict(pstB, "v")
        dummy_mms(6, huT[0], psA, "psA0")
        pass2(hvT, fvT, sm["vw2t"], sm["bfac"], sm["dwt"], v_out, "v", nc.scalar)
        dummy_mms(5, huT[0], psA, "psA0")
        ps2.release()

    nc.compile()
    return nc


_CACHE = {}


def _get_program():
    if "nc" not in _CACHE:
        _CACHE["nc"] = build_program()
    return _CACHE["nc"]


def make_in_maps(inputs):
    import ml_dtypes

    bf16 = ml_dtypes.bfloat16
    f8 = ml_dtypes.float8_e4m3

    adj = np.asarray(inputs["adj_matrix"], dtype=np.int32)
    u_sf = np.asarray(inputs["u_sideFeat"], dtype=np.float32)
    v_sf = np.asarray(inputs["v_sideFeat"], dtype=np.float32)
    msg_W = np.asarray(inputs["msg_W"], dtype=np.float64)
    dense_W = np.asarray(inputs["dense_W"], dtype=np.float32)
    u_W1 = np.asarray(inputs["u_W1"], dtype=np.float32)
    u_b1 = np.asarray(inputs["u_b1"], dtype=np.float32)
    u_W2 = np.asarray(inputs["u_W2"], dtype=np.float32)
    v_W1 = np.asarray(inputs["v_W1"], dtype=np.float32)
    v_b1 = np.asarray(inputs["v_b1"], dtype=np.float32)
    v_W2 = np.asarray(inputs["v_W2"], dtype=np.float32)

    # degree normalization (exact, f64); Csafe guard only matters off-support
    nz = adj != 0
    a = 1.0 / np.sqrt(np.maximum(nz.sum(axis=1), 1))  # [NU]
    b = 1.0 / np.sqrt(np.maximum(nz.sum(axis=0), 1))  # [NI]

    # fp8 stream weights. M_4 is eliminated via
    #   sum_r M_r W_r = A (W_4/4) + sum_{r!=4} M_r (W_r - r/4 W_4)
    # then the contraction-side degree factor and S are folded in, and the
    # result laid out k-copy-major for the DoubleRow lhsT APs:
    #   [p, k, s*256 + m] = T_s^T[128k+p, m],  s order [A, M1, M2, M3, M5]
    def w_stream(wT, fold):  # wT [R, M, 4000] -> [128, KT, WREC] fp8
        t = np.empty((NS, M, NU), np.float64)
        w4 = wT[3]
        t[0] = w4 / 4.0
        for si, r in enumerate((1, 2, 3, 5)):
            t[1 + si] = wT[r - 1] - (r / 4.0) * w4
        t *= fold[None, None, :] * SSCALE
        tp = np.zeros((NP, NS * M), np.float64)
        tp[:NU, :] = t.transpose(2, 0, 1).reshape(NU, NS * M)
        return np.ascontiguousarray(
            tp.reshape(KT, 128, WREC).transpose(1, 0, 2)
        ).astype(f8)

    wi_s = w_stream(msg_W[:, :, NU:], b)
    wu_s = w_stream(msg_W[:, :, :NU], a)

    # adjacency / M5 blocks, padded, k-copy-major: [128, KT, BP] per core
    def blocks(m):  # m [4000(contraction), 4000(moving)] -> [128, KT, 4000]
        mp = np.zeros((NP, NU), np.float32)
        mp[:NU, :] = m
        return mp.reshape(KT, 128, NU).transpose(1, 0, 2)

    adjv_b = blocks(adj)  # contraction over users (rows)
    adjtu_b = blocks(adj.T)  # contraction over items
    m5v_b = blocks((adj == 5).astype(np.float32))
    m5tu_b = blocks((adj.T == 5).astype(np.float32))

    def core_slice(bl, s):  # [128, KT, 4000] -> [128, KT, BP] fp8
        out = np.zeros((128, KT, BP), f8)
        out[:, :, :B] = bl[:, :, s : s + B].astype(f8)
        return out

    # batched side-weight tensors (bf16 at 128 partitions + f32 scalars)
    uftT = u_sf.T.astype(bf16)  # [FDIM, NU]
    vftT = v_sf.T.astype(bf16)
    sbf_shared = np.zeros((128, SBF_F), bf16)
    sbf_shared[:FDIM, SBF_UW1 : SBF_UW1 + SIDE] = u_W1.T.astype(bf16)
    sbf_shared[:FDIM, SBF_VW1 : SBF_VW1 + SIDE] = v_W1.T.astype(bf16)
    dwt = (dense_W.T / SSCALE).astype(bf16)  # [M, OUT]
    sbf_shared[:, SBF_DW0 : SBF_DW0 + OUT] = dwt[:128]
    sbf_shared[:, SBF_DW1 : SBF_DW1 + OUT] = dwt[128:]
    sbf_shared[:SIDE, SBF_UW2 : SBF_UW2 + OUT] = u_W2.T.astype(bf16)
    sbf_shared[:SIDE, SBF_VW2 : SBF_VW2 + OUT] = v_W2.T.astype(bf16)

    def chunked(v):  # [B] f64 -> [CH, 4] f32 column-per-chunk
        return np.ascontiguousarray(v.reshape(4, CH).T).astype(np.float32)

    in_maps = []
    for c in range(NCORES):
        s = c * B
        sbf = sbf_shared.copy()
        sbf[:FDIM, SBF_UFT : SBF_UFT + B] = uftT[:, s : s + B]
        sbf[:FDIM, SBF_VFT : SBF_VFT + B] = vftT[:, s : s + B]
        sfp = np.zeros((128, SFP_F), np.float32)
        sfp[:SIDE, 0] = u_b1
        sfp[:SIDE, 1] = v_b1
        sfp[:CH, 2:6] = chunked(a[s : s + B])
        sfp[:CH, 6:10] = chunked(b[s : s + B])
        in_maps.append(
            {
                "wi": wi_s,
                "wu": wu_s,
                "adjtu": core_slice(adjtu_b, s),
                "adjv": core_slice(adjv_b, s),
                "m5tu": core_slice(m5tu_b, s),
                "m5v": core_slice(m5v_b, s),
                "sbf": sbf,
                "sfp": sfp,
            }
        )
    return in_maps


def assemble(results):
    U = np.empty((NU, OUT), np.float32)
    V = np.empty((NI, OUT), np.float32)
    for c in range(NCORES):
        U[c * B : (c + 1) * B] = (
            results[c]["u_out"].transpose(1, 0, 2).reshape(B, OUT)
        )
        V[c * B : (c + 1) * B] = (
            results[c]["v_out"].transpose(1, 0, 2).reshape(B, OUT)
        )
    return (U, V)


def kernel(**inputs):
    from concourse.bass_utils import run_bass_kernel_spmd

    nc = _get_program()
    res = run_bass_kernel_spmd(nc, make_in_maps(inputs), core_ids=list(range(NCORES)))
    return assemble(res.results)
